# revision 39
# baseline (speedup 1.0000x reference)
"""Trainium2 Bass kernel for the per-game CriticNetwork (MoE-routed MLP).

Network (per sample b, with game g = idx[b]):
    h1  = relu(W1[g] @ state[b] + b1[g])          # [600]
    h2  = W2s @ h1 + b2s + W2a[g] @ action[b]     # [500]
    q   = W3[g] . relu(h2) + b3[g]                # scalar

Strategy: all MoE routing happens on the HOST. idx is (stably) sorted into
per-game contiguous segments, each segment is padded up to 512-sample tiles,
and the tile list is padded to a fixed 72 tiles (9 per core x 8 cores).
Every tile is single-game, so the device kernel is a fully static dense
pipeline; the host pre-gathers per-tile weight views (pre-transposed for the
PE's lhsT layout) so the device does zero routing and zero transposes.

Device per tile t (512 samples, fp32 everywhere, PSUM accumulation):
    L1: 5 matmuls  [K=128(d), M=128(h1 chunk), N=512(b)] + fused relu+bias (ACT)
    L2: 4 m-chunks x (5 K=128 matmuls of shared W2s + 1 K=16 matmul of W2a[g])
        accumulated in PSUM + fused relu+bias (ACT)
    L3: 4 accumulating matmuls [K=128, M=1, N=512] with W3[g] -> q[1, 512]
b3 is added on the host after gathering.
"""

import numpy as np

import concourse.bass as bass
import concourse.mybir as mybir
import concourse.tile as tile
from concourse import bacc
from concourse.bass import ts
from concourse.bass_utils import run_bass_kernel_spmd

F32 = mybir.dt.float32
RELU = mybir.ActivationFunctionType.Relu

# Matmul operand dtype: bfloat16 runs the PE at 1 cycle/row (fp32 is 4).
MM_DT = mybir.dt.bfloat16
_NP_MM_DT = mybir.dt.np(MM_DT)

# Engine-assignment / buffering knobs (tuned via dev_sweep.py).
CFG = {
    "l1_dve": (),        # L1 relu chunks done on DVE (rest on ACT)
    "hf_dve": (0, 1, 2, 3),  # L2 relu chunks done on DVE (rest on ACT)
    "q_dve": True,       # q psum->sbuf copy on DVE (else ACT)
    "ps1_bufs": 3,
    "psq_bufs": 1,
    "w2a_last": True,    # W2a matmuls close the ps2 group (else open it)
}

G = 8          # games
D = 128        # state dim
A = 16         # action dim
H1 = 600       # hidden 1 (padded to 640 = 5 * 128)
H2 = 500       # hidden 2 (padded to 512 = 4 * 128)
B = 32768      # batch
H1P, H2P = 640, 512
K1 = H1P // 128   # 5 h1 chunks
M2 = H2P // 128   # 4 h2 chunks
T = 512        # samples per tile (one PSUM bank of fp32)
NCORES = 8
NT = 9         # tiles per core; 72 total >= 64 + 7 worst-case segment padding
BPC = NT * T   # 4608 lanes per core

_NC = None


def build_nc():
    nc = bacc.Bacc("TRN2", target_bir_lowering=False, debug=False,
                   num_devices=NCORES)

    # Per-tile packed weight blob [128, WB] (bf16):
    #   cols 0:640        w1t   (W1[g].T, d x h1p)
    #   cols 640:645      b1t   (bias chunk c at col 640+c)
    #   cols 645:649      w3t   (W3[g] chunk m at col 645+m)
    #   cols 649:777      w2at4 (W2a[g].T row-packed: chunk m at rows 32m..+15)
    WB = H1P + K1 + M2 + 128
    stateT = nc.declare_dram_parameter("stateT", [D, BPC], MM_DT, isOutput=False)
    # action, replicated on partition blocks {32m..32m+15} so the four W2a
    # matmuls can run as concurrent 32-row PE tiles (tile_position row packing)
    at4 = nc.declare_dram_parameter("aT", [128, BPC], MM_DT, isOutput=False)
    wblob = nc.declare_dram_parameter("wblob", [NT, 128, WB], MM_DT,
                                      isOutput=False)
    b1t = nc.declare_dram_parameter("b1t", [NT, 128, K1], F32, isOutput=False)
    w2st = nc.declare_dram_parameter("w2st", [H1P, H2P], MM_DT, isOutput=False)
    b2st = nc.declare_dram_parameter("b2st", [128, M2], F32, isOutput=False)
    q = nc.declare_dram_parameter("q", [1, BPC], F32, isOutput=True)

    with tile.TileContext(nc) as tc:
        with (
            tc.tile_pool(name="const", bufs=1) as const,
            tc.tile_pool(name="wts", bufs=4) as wts,
            tc.tile_pool(name="acts", bufs=4) as acts,
            tc.tile_pool(name="hpool", bufs=3) as hpool,
            tc.tile_pool(name="outp", bufs=2) as outp,
            tc.tile_pool(name="ps1", bufs=CFG["ps1_bufs"], space="PSUM") as ps1p,
            tc.tile_pool(name="ps2", bufs=1, space="PSUM") as ps2p,
            tc.tile_pool(name="psq", bufs=CFG["psq_bufs"], space="PSUM") as psqp,
        ):
            # Shared weights, loaded once (emitted after tile 0's loads so
            # the first L1 matmul's inputs get the DMA queues first).
            w2st_sb = const.tile([128, K1, H2P], MM_DT)
            b2st_sb = const.tile([128, M2], F32)

            def load_const():
                nc.sync.dma_start(
                    w2st_sb[:], w2st.ap().rearrange("(c p) n -> p c n", p=128))
                nc.sync.dma_start(b2st_sb[:], b2st.ap())

            def load_tile(t):
                wb = wts.tile([128, WB], MM_DT, tag="wb")
                nc.sync.dma_start(wb[:], wblob[t])
                b1 = wts.tile([128, K1], F32, tag="b1")
                nc.sync.dma_start(b1[:], b1t[t])
                st = acts.tile([D, T], MM_DT, tag="st")
                nc.sync.dma_start(st[:], stateT[:, ts(t, T)])
                at = acts.tile([128, T], MM_DT, tag="at")
                nc.sync.dma_start(at[:], at4[:, ts(t, T)])
                return wb, b1, st, at

            def relu_op(out, in_, bias, on_dve):
                if on_dve:
                    nc.vector.tensor_scalar(out, in_, bias, 0.0,
                                            mybir.AluOpType.add,
                                            mybir.AluOpType.max)
                else:
                    nc.scalar.activation(out, in_, RELU, bias=bias)

            def l1_relu(h1, ps1, b1, c):
                relu_op(h1[:, c, :], ps1[:], b1[:, c:c + 1],
                        c in CFG["l1_dve"])

            # Hand-scheduled PE order per tile (PE executes in-order, so
            # emission order decides what runs while relus drain):
            #   L1 c0..c2 | L3(t-1) m0,m1 | L1 c3 | L3(t-1) m2,m3 | L1 c4 |
            #   W2s c-outer/m-inner (c0 group opens ps2) | W2a x4 (closers)
            # c-outer L2 gives the PE four ready matmuls per arriving h1
            # chunk, faster than the relu production cadence; L3 matmuls of
            # the previous tile fill L1's psum-slot waits.
            def emit_l1_chunk(h1, wb, b1, st, c):
                ps1 = ps1p.tile([128, T], F32, tag="ps1")
                nc.tensor.matmul(ps1[:], wb[:, ts(c, 128)], st[:],
                                 start=True, stop=True)
                l1_relu(h1, ps1, b1, c)

            def emit_l3_mm(prev, m):
                tp, wbp, hfp, psqp_t = prev
                nc.tensor.matmul(psqp_t[:], wbp[:, 645 + m:646 + m],
                                 hfp[:, m, :], start=(m == 0),
                                 stop=(m == M2 - 1))

            def emit_l3_out(prev):
                tp, wbp, hfp, psq = prev
                q_sb = outp.tile([1, T], F32, tag="q")
                if CFG["q_dve"]:
                    nc.vector.tensor_copy(q_sb[:], psq[:])
                else:
                    nc.scalar.activation(q_sb[:], psq[:],
                                         mybir.ActivationFunctionType.Copy)
                nc.sync.dma_start(q[0:1, ts(tp, T)], q_sb[:])

            pend = None   # (t, wb, hf, psq)
            for t in range(NT):
                wb, b1, st, at = load_tile(t)
                if t == 0:
                    load_const()

                h1 = hpool.tile([128, K1, T], MM_DT, tag="h1")
                if pend is not None:
                    psq_t = psqp.tile([1, T], F32, tag="psq")
                    pend = (pend[0], pend[1], pend[2], psq_t)
                for c in range(3):
                    emit_l1_chunk(h1, wb, b1, st, c)
                if pend is not None:
                    emit_l3_mm(pend, 0)
                    emit_l3_mm(pend, 1)
                emit_l1_chunk(h1, wb, b1, st, 3)
                if pend is not None:
                    emit_l3_mm(pend, 2)
                    emit_l3_mm(pend, 3)
                emit_l1_chunk(h1, wb, b1, st, 4)
                if pend is not None:
                    emit_l3_out(pend)
                    pend = None

                ps2 = ps2p.tile([128, M2, T], F32, tag="ps2")
                hf = hpool.tile([128, M2, T], MM_DT, tag="hf")
                for c in range(K1 - 1):
                    for m in range(M2):
                        nc.tensor.matmul(ps2[:, m, :],
                                         w2st_sb[:, c, ts(m, 128)],
                                         h1[:, c, :], start=(c == 0),
                                         stop=False)
                # W2a before the last c-group: keeps the four row-tiled
                # matmuls adjacent (HW concurrency) while letting each bank
                # finish, and its relu start, as early as possible.
                w2a_v = wb[:, 649:649 + 128].rearrange("(a b) f -> a b f",
                                                       b=32)
                at_v = at[:].rearrange("(a b) f -> a b f", b=32)
                for m in range(M2):
                    nc.tensor.matmul(ps2[:, m, :], w2a_v[m, 0:A, :],
                                     at_v[m, 0:A, :], start=False, stop=False,
                                     tile_position=(32 * m, 0))
                for m in range(M2):
                    nc.tensor.matmul(ps2[:, m, :],
                                     w2st_sb[:, K1 - 1, ts(m, 128)],
                                     h1[:, K1 - 1, :], start=False, stop=True)
                for m in range(M2):
                    relu_op(hf[:, m, :], ps2[:, m, :], b2st_sb[:, m:m + 1],
                            m in CFG["hf_dve"])
                pend = (t, wb, hf, None)
            psq_t = psqp.tile([1, T], F32, tag="psq")
            pend = (pend[0], pend[1], pend[2], psq_t)
            for m in range(M2):
                emit_l3_mm(pend, m)
            emit_l3_out(pend)

    nc.compile()
    return nc


def _get_nc():
    global _NC
    if _NC is None:
        _NC = build_nc()
    return _NC


def _plan_tiles(idx):
    """Stable-sort samples by game, pad each game segment to 512-sample
    tiles, pad the tile list to the fixed 72. Returns (sel, valid, gids):
    sel[t, l] = original sample index feeding lane l of tile t."""
    perm = np.argsort(idx, kind="stable")
    counts = np.bincount(idx, minlength=G)
    ntot = NCORES * NT
    sel = np.zeros((ntot, T), np.int64)
    valid = np.zeros((ntot, T), bool)
    gids = np.zeros(ntot, np.int64)
    pos, t = 0, 0
    for g in range(G):
        cg = int(counts[g])
        for k in range((cg + T - 1) // T):
            n = min(T, cg - k * T)
            lanes = perm[pos:pos + n]
            sel[t, :n] = lanes
            valid[t, :n] = True
            if n < T:
                sel[t, n:] = lanes[0]
            gids[t] = g
            pos += n
            t += 1
    assert t <= ntot, f"tile plan overflow: {t} > {ntot}"
    return sel, valid, gids


def build_in_maps(inputs):
    state = np.ascontiguousarray(np.asarray(inputs["state"], np.float32))
    action = np.ascontiguousarray(np.asarray(inputs["action"], np.float32))
    idx = np.asarray(inputs["idx"]).astype(np.int64)
    W1 = np.asarray(inputs["W1"], np.float32)
    b1 = np.asarray(inputs["b1"], np.float32)
    W2s = np.asarray(inputs["W2s"], np.float32)
    b2s = np.asarray(inputs["b2s"], np.float32)
    W2a = np.asarray(inputs["W2a"], np.float32)
    W3 = np.asarray(inputs["W3"], np.float32)
    assert state.shape == (B, D) and action.shape == (B, A)

    sel, valid, gids = _plan_tiles(idx)

    # Pre-transposed / padded weight views, indexed per tile by game id.
    W1T_all = np.zeros((G, D, H1P), np.float32)
    W1T_all[:, :, :H1] = W1.transpose(0, 2, 1)
    b1P = np.zeros((G, H1P), np.float32)
    b1P[:, :H1] = b1
    b1c_all = np.ascontiguousarray(b1P.reshape(G, K1, 128).transpose(0, 2, 1))
    W2sTP = np.zeros((H1P, H2P), np.float32)
    W2sTP[:H1, :H2] = W2s.T
    W2aT_all = np.zeros((G, A, H2P), np.float32)
    W2aT_all[:, :, :H2] = W2a.transpose(0, 2, 1)
    # Row-packed layout for tile_position: chunk m at partitions 32m..32m+15
    W2aT4_all = np.zeros((G, 128, 128), np.float32)
    for m in range(M2):
        W2aT4_all[:, 32 * m:32 * m + A, :] = \
            W2aT_all[:, :, 128 * m:128 * (m + 1)]
    b2sP = np.zeros(H2P, np.float32)
    b2sP[:H2] = b2s
    b2st = np.ascontiguousarray(b2sP.reshape(M2, 128).T)
    W3P = np.zeros((G, H2P), np.float32)
    W3P[:, :H2] = W3
    W3T_all = np.ascontiguousarray(W3P.reshape(G, M2, 128).transpose(0, 2, 1))

    # Per-game packed weight blob (layout documented in build_nc).
    WB = H1P + K1 + M2 + 128
    blob_all = np.zeros((G, 128, WB), np.float32)
    blob_all[:, :, 0:H1P] = W1T_all
    blob_all[:, :, H1P:H1P + K1] = b1c_all
    blob_all[:, :, H1P + K1:H1P + K1 + M2] = W3T_all
    blob_all[:, :, H1P + K1 + M2:] = W2aT4_all
    blob_all = blob_all.astype(_NP_MM_DT)

    in_maps = []
    for c in range(NCORES):
        tsl = slice(c * NT, (c + 1) * NT)
        lanes = sel[tsl].reshape(-1)
        gt = gids[tsl]
        aTc = np.ascontiguousarray(action[lanes].T)          # [16, BPC]
        at4 = np.zeros((128, BPC), np.float32)
        for m in range(M2):
            at4[32 * m:32 * m + A] = aTc
        in_maps.append({
            "stateT": np.ascontiguousarray(state[lanes].T).astype(_NP_MM_DT),
            "aT": at4.astype(_NP_MM_DT),
            "wblob": np.ascontiguousarray(blob_all[gt]),
            "b1t": np.ascontiguousarray(b1c_all[gt]),
            "w2st": W2sTP.astype(_NP_MM_DT),
            "b2st": b2st,
        })
    return in_maps, sel, valid


def kernel(**inputs):
    idx = np.asarray(inputs["idx"]).astype(np.int64)
    b3 = np.asarray(inputs["b3"], np.float32)
    in_maps, sel, valid = build_in_maps(inputs)

    res = run_bass_kernel_spmd(_get_nc(), in_maps, list(range(NCORES))).results
    qv = np.concatenate([np.asarray(res[c]["q"]).reshape(-1)
                         for c in range(NCORES)])

    out = np.zeros(B, np.float32)
    flat_sel = sel.reshape(-1)
    flat_valid = valid.reshape(-1)
    out[flat_sel[flat_valid]] = qv[flat_valid]
    out += b3[idx]
    return out.astype(np.float32)


# revision 45
# speedup vs baseline: 1.2113x; 1.2113x over previous
"""Trainium2 Bass kernel for the per-game CriticNetwork (MoE-routed MLP).

Network (per sample b, with game g = idx[b]):
    h1  = relu(W1[g] @ state[b] + b1[g])          # [600]
    h2  = W2s @ h1 + b2s + W2a[g] @ action[b]     # [500]
    q   = W3[g] . relu(h2) + b3[g]                # scalar

Strategy: all MoE routing happens on the HOST. idx is (stably) sorted into
per-game contiguous segments, each segment is padded up to 512-sample tiles,
and the tile list is padded to a fixed 72 tiles (9 per core x 8 cores).
Every tile is single-game, so the device kernel is a fully static dense
pipeline; the host pre-gathers per-tile weight views (pre-transposed for the
PE's lhsT layout) so the device does zero routing and zero transposes.

Device per tile t (512 samples, fp32 everywhere, PSUM accumulation):
    L1: 5 matmuls  [K=128(d), M=128(h1 chunk), N=512(b)] + fused relu+bias (ACT)
    L2: 4 m-chunks x (5 K=128 matmuls of shared W2s + 1 K=16 matmul of W2a[g])
        accumulated in PSUM + fused relu+bias (ACT)
    L3: 4 accumulating matmuls [K=128, M=1, N=512] with W3[g] -> q[1, 512]
b3 is added on the host after gathering.
"""

import numpy as np

import concourse.bass as bass
import concourse.mybir as mybir
import concourse.tile as tile
from concourse import bacc
from concourse.bass import ts
from concourse.bass_utils import run_bass_kernel_spmd

F32 = mybir.dt.float32
RELU = mybir.ActivationFunctionType.Relu

# Matmul operand dtype: bfloat16 runs the PE at 1 cycle/row (fp32 is 4).
MM_DT = mybir.dt.bfloat16
_NP_MM_DT = mybir.dt.np(MM_DT)

# Engine-assignment / buffering knobs (tuned via dev_sweep.py).
CFG = {
    "l1_dve": (),        # L1 relu chunks done on DVE (rest on ACT)
    "hf_dve": (0, 1, 2, 3),  # L2 relu chunks done on DVE (rest on ACT)
    "q_dve": False,      # q psum->sbuf copy on DVE (else ACT)
    "ps1_bufs": 4,
    "psq_bufs": 1,
    "l2_m_outer": True,  # m-outer L2 with per-m psum tiles (else c-outer)
    "ps2m_bufs": 3,
}

G = 8          # games
D = 128        # state dim
A = 16         # action dim
H1 = 600       # hidden 1 (padded to 640 = 5 * 128)
H2 = 500       # hidden 2 (padded to 512 = 4 * 128)
B = 32768      # batch
H1P, H2P = 640, 512
K1 = H1P // 128   # 5 h1 chunks
M2 = H2P // 128   # 4 h2 chunks
T = 512        # samples per tile (one PSUM bank of fp32)
NCORES = 8
NT = 9         # tiles per core; 72 total >= 64 + 7 worst-case segment padding
BPC = NT * T   # 4608 lanes per core

_NC = None


def build_nc():
    nc = bacc.Bacc("TRN2", target_bir_lowering=False, debug=False,
                   num_devices=NCORES)

    # Per-tile packed weight blob [128, WB] (bf16):
    #   cols 0:640        w1t   (W1[g].T, d x h1p)
    #   cols 640:645      b1t   (bias chunk c at col 640+c)
    #   cols 645:649      w3t   (W3[g] chunk m at col 645+m)
    #   cols 649:777      w2at4 (W2a[g].T row-packed: chunk m at rows 32m..+15)
    WB = H1P + K1 + M2 + 128
    stateT = nc.declare_dram_parameter("stateT", [D, BPC], MM_DT, isOutput=False)
    # action, replicated on partition blocks {32m..32m+15} so the four W2a
    # matmuls can run as concurrent 32-row PE tiles (tile_position row packing)
    at4 = nc.declare_dram_parameter("aT", [128, BPC], MM_DT, isOutput=False)
    wblob = nc.declare_dram_parameter("wblob", [NT, 128, WB], MM_DT,
                                      isOutput=False)
    b1t = nc.declare_dram_parameter("b1t", [NT, 128, K1], F32, isOutput=False)
    w2st = nc.declare_dram_parameter("w2st", [H1P, H2P], MM_DT, isOutput=False)
    b2st = nc.declare_dram_parameter("b2st", [128, M2], F32, isOutput=False)
    q = nc.declare_dram_parameter("q", [1, BPC], F32, isOutput=True)

    with tile.TileContext(nc) as tc:
        with (
            tc.tile_pool(name="const", bufs=1) as const,
            tc.tile_pool(name="wts", bufs=4) as wts,
            tc.tile_pool(name="acts", bufs=4) as acts,
            tc.tile_pool(name="hpool", bufs=3) as hpool,
            tc.tile_pool(name="outp", bufs=2) as outp,
            tc.tile_pool(name="ps1", bufs=CFG["ps1_bufs"], space="PSUM") as ps1p,
            tc.tile_pool(name="ps2",
                         bufs=CFG["ps2m_bufs"] if CFG["l2_m_outer"] else 1,
                         space="PSUM") as ps2p,
            tc.tile_pool(name="psq", bufs=CFG["psq_bufs"], space="PSUM") as psqp,
        ):
            # Shared weights, loaded once (emitted after tile 0's loads so
            # the first L1 matmul's inputs get the DMA queues first).
            w2st_sb = const.tile([128, K1, H2P], MM_DT)
            b2st_sb = const.tile([128, M2], F32)

            def load_const():
                nc.sync.dma_start(
                    w2st_sb[:], w2st.ap().rearrange("(c p) n -> p c n", p=128))
                nc.sync.dma_start(b2st_sb[:], b2st.ap())

            def load_tile(t):
                wb = wts.tile([128, WB], MM_DT, tag="wb")
                nc.sync.dma_start(wb[:], wblob[t])
                b1 = wts.tile([128, K1], F32, tag="b1")
                nc.sync.dma_start(b1[:], b1t[t])
                st = acts.tile([D, T], MM_DT, tag="st")
                nc.sync.dma_start(st[:], stateT[:, ts(t, T)])
                at = acts.tile([128, T], MM_DT, tag="at")
                nc.sync.dma_start(at[:], at4[:, ts(t, T)])
                return wb, b1, st, at

            def relu_op(out, in_, bias, on_dve):
                if on_dve:
                    nc.vector.tensor_scalar(out, in_, bias, 0.0,
                                            mybir.AluOpType.add,
                                            mybir.AluOpType.max)
                else:
                    nc.scalar.activation(out, in_, RELU, bias=bias)

            def l1_relu(h1, ps1, b1, c):
                relu_op(h1[:, c, :], ps1[:], b1[:, c:c + 1],
                        c in CFG["l1_dve"])

            # Hand-scheduled PE order per tile (PE executes in-order, so
            # emission order decides what runs while relus drain):
            #   L1 c0..c2 | L3(t-1) m0,m1 | L1 c3 | L3(t-1) m2,m3 | L1 c4 |
            #   W2s c-outer/m-inner (c0 group opens ps2) | W2a x4 (closers)
            # c-outer L2 gives the PE four ready matmuls per arriving h1
            # chunk, faster than the relu production cadence; L3 matmuls of
            # the previous tile fill L1's psum-slot waits.
            def emit_l1_chunk(h1, wb, b1, st, c):
                ps1 = ps1p.tile([128, T], F32, tag="ps1")
                nc.tensor.matmul(ps1[:], wb[:, ts(c, 128)], st[:],
                                 start=True, stop=True)
                l1_relu(h1, ps1, b1, c)

            def emit_l3_mm(prev, m):
                tp, wbp, hfp, psqp_t = prev
                nc.tensor.matmul(psqp_t[:], wbp[:, 645 + m:646 + m],
                                 hfp[:, m, :], start=(m == 0),
                                 stop=(m == M2 - 1))

            def emit_l3_out(prev):
                tp, wbp, hfp, psq = prev
                q_sb = outp.tile([1, T], F32, tag="q")
                if CFG["q_dve"]:
                    nc.vector.tensor_copy(q_sb[:], psq[:])
                else:
                    nc.scalar.activation(q_sb[:], psq[:],
                                         mybir.ActivationFunctionType.Copy)
                nc.sync.dma_start(q[0:1, ts(tp, T)], q_sb[:])

            pend = None   # (t, wb, hf, psq)
            for t in range(NT):
                wb, b1, st, at = load_tile(t)
                if t == 0:
                    load_const()

                h1 = hpool.tile([128, K1, T], MM_DT, tag="h1")
                if pend is not None:
                    psq_t = psqp.tile([1, T], F32, tag="psq")
                    pend = (pend[0], pend[1], pend[2], psq_t)
                for c in range(3):
                    emit_l1_chunk(h1, wb, b1, st, c)
                if pend is not None:
                    emit_l3_mm(pend, 0)
                    emit_l3_mm(pend, 1)
                emit_l1_chunk(h1, wb, b1, st, 3)
                if pend is not None:
                    emit_l3_mm(pend, 2)
                    emit_l3_mm(pend, 3)
                emit_l1_chunk(h1, wb, b1, st, 4)
                if pend is not None:
                    emit_l3_out(pend)
                    pend = None

                hf = hpool.tile([128, M2, T], MM_DT, tag="hf")
                w2a_v = wb[:, 649:649 + 128].rearrange("(a b) f -> a b f",
                                                       b=32)
                at_v = at[:].rearrange("(a b) f -> a b f", b=32)
                if CFG["l2_m_outer"]:
                    # m-outer: per-m psum tiles, relu drains pipelined.
                    for m in range(M2):
                        ps2m = ps2p.tile([128, T], F32, tag="ps2")
                        for c in range(K1):
                            nc.tensor.matmul(ps2m[:],
                                             w2st_sb[:, c, ts(m, 128)],
                                             h1[:, c, :], start=(c == 0),
                                             stop=False)
                        nc.tensor.matmul(ps2m[:], w2a_v[m, 0:A, :],
                                         at_v[m, 0:A, :], start=False,
                                         stop=True, tile_position=(32 * m, 0))
                        relu_op(hf[:, m, :], ps2m[:], b2st_sb[:, m:m + 1],
                                m in CFG["hf_dve"])
                else:
                    ps2 = ps2p.tile([128, M2, T], F32, tag="ps2")
                    for c in range(K1 - 1):
                        for m in range(M2):
                            nc.tensor.matmul(ps2[:, m, :],
                                             w2st_sb[:, c, ts(m, 128)],
                                             h1[:, c, :], start=(c == 0),
                                             stop=False)
                    # W2a before the last c-group: keeps the four row-tiled
                    # matmuls adjacent (HW concurrency) while letting each
                    # bank finish, and its relu start, as early as possible.
                    for m in range(M2):
                        nc.tensor.matmul(ps2[:, m, :], w2a_v[m, 0:A, :],
                                         at_v[m, 0:A, :], start=False,
                                         stop=False,
                                         tile_position=(32 * m, 0))
                    for m in range(M2):
                        nc.tensor.matmul(ps2[:, m, :],
                                         w2st_sb[:, K1 - 1, ts(m, 128)],
                                         h1[:, K1 - 1, :], start=False,
                                         stop=True)
                    for m in range(M2):
                        relu_op(hf[:, m, :], ps2[:, m, :],
                                b2st_sb[:, m:m + 1], m in CFG["hf_dve"])
                pend = (t, wb, hf, None)
            psq_t = psqp.tile([1, T], F32, tag="psq")
            pend = (pend[0], pend[1], pend[2], psq_t)
            for m in range(M2):
                emit_l3_mm(pend, m)
            emit_l3_out(pend)

    nc.compile()
    return nc


def _get_nc():
    global _NC
    if _NC is None:
        _NC = build_nc()
    return _NC


def _plan_tiles(idx):
    """Stable-sort samples by game, pad each game segment to 512-sample
    tiles, pad the tile list to the fixed 72. Returns (sel, valid, gids):
    sel[t, l] = original sample index feeding lane l of tile t."""
    perm = np.argsort(idx, kind="stable")
    counts = np.bincount(idx, minlength=G)
    ntot = NCORES * NT
    sel = np.zeros((ntot, T), np.int64)
    valid = np.zeros((ntot, T), bool)
    gids = np.zeros(ntot, np.int64)
    pos, t = 0, 0
    for g in range(G):
        cg = int(counts[g])
        for k in range((cg + T - 1) // T):
            n = min(T, cg - k * T)
            lanes = perm[pos:pos + n]
            sel[t, :n] = lanes
            valid[t, :n] = True
            if n < T:
                sel[t, n:] = lanes[0]
            gids[t] = g
            pos += n
            t += 1
    assert t <= ntot, f"tile plan overflow: {t} > {ntot}"
    return sel, valid, gids


def build_in_maps(inputs):
    state = np.ascontiguousarray(np.asarray(inputs["state"], np.float32))
    action = np.ascontiguousarray(np.asarray(inputs["action"], np.float32))
    idx = np.asarray(inputs["idx"]).astype(np.int64)
    W1 = np.asarray(inputs["W1"], np.float32)
    b1 = np.asarray(inputs["b1"], np.float32)
    W2s = np.asarray(inputs["W2s"], np.float32)
    b2s = np.asarray(inputs["b2s"], np.float32)
    W2a = np.asarray(inputs["W2a"], np.float32)
    W3 = np.asarray(inputs["W3"], np.float32)
    assert state.shape == (B, D) and action.shape == (B, A)

    sel, valid, gids = _plan_tiles(idx)

    # Pre-transposed / padded weight views, indexed per tile by game id.
    W1T_all = np.zeros((G, D, H1P), np.float32)
    W1T_all[:, :, :H1] = W1.transpose(0, 2, 1)
    b1P = np.zeros((G, H1P), np.float32)
    b1P[:, :H1] = b1
    b1c_all = np.ascontiguousarray(b1P.reshape(G, K1, 128).transpose(0, 2, 1))
    W2sTP = np.zeros((H1P, H2P), np.float32)
    W2sTP[:H1, :H2] = W2s.T
    W2aT_all = np.zeros((G, A, H2P), np.float32)
    W2aT_all[:, :, :H2] = W2a.transpose(0, 2, 1)
    # Row-packed layout for tile_position: chunk m at partitions 32m..32m+15
    W2aT4_all = np.zeros((G, 128, 128), np.float32)
    for m in range(M2):
        W2aT4_all[:, 32 * m:32 * m + A, :] = \
            W2aT_all[:, :, 128 * m:128 * (m + 1)]
    b2sP = np.zeros(H2P, np.float32)
    b2sP[:H2] = b2s
    b2st = np.ascontiguousarray(b2sP.reshape(M2, 128).T)
    W3P = np.zeros((G, H2P), np.float32)
    W3P[:, :H2] = W3
    W3T_all = np.ascontiguousarray(W3P.reshape(G, M2, 128).transpose(0, 2, 1))

    # Per-game packed weight blob (layout documented in build_nc).
    WB = H1P + K1 + M2 + 128
    blob_all = np.zeros((G, 128, WB), np.float32)
    blob_all[:, :, 0:H1P] = W1T_all
    blob_all[:, :, H1P:H1P + K1] = b1c_all
    blob_all[:, :, H1P + K1:H1P + K1 + M2] = W3T_all
    blob_all[:, :, H1P + K1 + M2:] = W2aT4_all
    blob_all = blob_all.astype(_NP_MM_DT)

    in_maps = []
    for c in range(NCORES):
        tsl = slice(c * NT, (c + 1) * NT)
        lanes = sel[tsl].reshape(-1)
        gt = gids[tsl]
        aTc = np.ascontiguousarray(action[lanes].T)          # [16, BPC]
        at4 = np.zeros((128, BPC), np.float32)
        for m in range(M2):
            at4[32 * m:32 * m + A] = aTc
        in_maps.append({
            "stateT": np.ascontiguousarray(state[lanes].T).astype(_NP_MM_DT),
            "aT": at4.astype(_NP_MM_DT),
            "wblob": np.ascontiguousarray(blob_all[gt]),
            "b1t": np.ascontiguousarray(b1c_all[gt]),
            "w2st": W2sTP.astype(_NP_MM_DT),
            "b2st": b2st,
        })
    return in_maps, sel, valid


def kernel(**inputs):
    idx = np.asarray(inputs["idx"]).astype(np.int64)
    b3 = np.asarray(inputs["b3"], np.float32)
    in_maps, sel, valid = build_in_maps(inputs)

    res = run_bass_kernel_spmd(_get_nc(), in_maps, list(range(NCORES))).results
    qv = np.concatenate([np.asarray(res[c]["q"]).reshape(-1)
                         for c in range(NCORES)])

    out = np.zeros(B, np.float32)
    flat_sel = sel.reshape(-1)
    flat_valid = valid.reshape(-1)
    out[flat_sel[flat_valid]] = qv[flat_valid]
    out += b3[idx]
    return out.astype(np.float32)
